# revision 30
# baseline (speedup 1.0000x reference)
"""Trainium2 Bass kernel for nn_CausalWanSelfAttention (sparse_attention).

Strategy: tensor-parallel over heads across 8 NeuronCores. Each core owns
2 of the 16 heads and processes all 1760 tokens:
  - fused QKV projection (bf16 matmuls, PSUM-accumulated over 16 k-tiles)
  - RMS-norm: local sum-of-squares, tiny AllReduce for the full-2048-channel
    statistic, ln/exp-based rsqrt on ACT
  - rope on DVE (free-dim channel pairs), PE-transpose of q/k per head
  - attention in transposed layout: scores^T = kw^T(T) @ rq^T, exp on ACT,
    PV accumulation on PE; softmax denominator via DVE accumulation +
    GPSIMD partition_all_reduce; normalize with a DVE divide
  - output projection, f32 ReduceScatter so each core emits 220 rows
Host side (free): input slicing/transposition/bf16 casts, rope freq table,
final concat + output bias.
"""
import sys

for _p in ("/opt/trn_rl_repo", "/root/.axon_site/_ro/trn_rl_repo"):
    if _p not in sys.path:
        sys.path.append(_p)

import numpy as np
import ml_dtypes

import concourse.bass as bass
import concourse.bacc as bacc
import concourse.mybir as mybir
from concourse import bass_isa
from concourse.tile import TileContext
from concourse.bass_utils import run_bass_kernel_spmd
from concourse.masks import make_identity

BF16 = ml_dtypes.bfloat16
S, DIM, NH, D = 1760, 2048, 16, 128
TW = 3520          # attention window length
WIN0 = 2640        # cache rows [2640:4400] form the first half of the window
NCORES, HPC = 8, 2
CH = HPC * D       # 256 channels per core
EPS = 1e-6
SCALE = 1.0 / float(np.sqrt(D))
S_OUT = S // NCORES  # 220 rows of output per core

S_TILES = [(i * 128, min(128, S - i * 128)) for i in range((S + 127) // 128)]
NT = len(S_TILES)  # 14
# window t-tiles: cache part [0,1760) then new part [1760,3520)
T_TILES = ([("c", j, off, sz) for j, (off, sz) in enumerate(S_TILES)]
           + [("n", j, off + S, sz) for j, (off, sz) in enumerate(S_TILES)])
SJ = [(0, 880), (880, 880)]  # attention s-chunks

_CACHE = {}


def _emit(nc):
    dt = mybir.dt
    BF, F32 = dt.bfloat16, dt.float32
    A = mybir.ActivationFunctionType
    Op = mybir.AluOpType
    core_ids = list(range(NCORES))

    xT = nc.declare_dram_parameter("xT", [NT, 128, DIM], BF, isOutput=False)
    wT = nc.declare_dram_parameter("wT", [DIM, 3 * CH], BF, isOutput=False)
    woT = nc.declare_dram_parameter("woT", [DIM, DIM], BF, isOutput=False)
    ckT = nc.declare_dram_parameter("ckT", [HPC, D, S], BF, isOutput=False)
    cv = nc.declare_dram_parameter("cv", [HPC, 128, NT * D], BF, isOutput=False)
    fr2d = nc.declare_dram_parameter("fr2", [S, 2 * 64], BF, isOutput=False)
    fi2d = nc.declare_dram_parameter("fi2", [S, 2 * 64], BF, isOutput=False)
    gqd = nc.declare_dram_parameter("gq", [1, CH], F32, isOutput=False)
    gkd = nc.declare_dram_parameter("gk", [1, CH], F32, isOutput=False)
    bqd = nc.declare_dram_parameter("bq", [1, CH], F32, isOutput=False)
    bkd = nc.declare_dram_parameter("bk", [1, CH], F32, isOutput=False)
    bvd = nc.declare_dram_parameter("bv", [1, CH], F32, isOutput=False)
    y_out = nc.declare_dram_parameter("y", [S_OUT, DIM], F32, isOutput=True)

    GSIZES = [5, 5, 4]
    ss_in = [nc.dram_tensor(f"ss_in{g}", [2, 128, GSIZES[g]], F32)
             for g in range(3)]
    ss_out = [nc.dram_tensor(f"ss_out{g}", [2, 128, GSIZES[g]], F32,
                             addr_space="Shared") for g in range(3)]
    # o-matrix all-to-all: two waves (s 0:880 and 880:1760); each core ends
    # up with o^T columns for its own 110-row slice of each wave
    a2a_in = [nc.dram_tensor(f"a2a_in{w}", [NCORES, CH, 110], BF)
              for w in range(2)]
    a2a_out = [nc.dram_tensor(f"a2a_out{w}", [NCORES, CH, 110], BF)
               for w in range(2)]

    from contextlib import ExitStack
    with TileContext(nc) as tc, ExitStack() as stack:
        cpool = stack.enter_context(tc.tile_pool(name="const", bufs=1))
        wpool = stack.enter_context(tc.tile_pool(name="work", bufs=3))
        ppool = tc.alloc_tile_pool(name="projp", bufs=1)

        # ---- constants ----
        ident = cpool.tile([128, 128], BF, tag="ident")
        make_identity(nc, ident[:])

        xt0 = wpool.tile([128, DIM], BF, tag="xt0", bufs=1, name="xt0pre")
        nc.sync.dma_start(out=xt0[:], in_=xT[0])
        wT_sb = []
        for kk in range(16):
            t = ppool.tile([128, 3 * CH], BF, tag=f"wT{kk}", name=f"wT{kk}")
            nc.sync.dma_start(out=t[:], in_=wT[128 * kk:128 * (kk + 1), :])
            wT_sb.append(t)

        def bcast_row(name, src):
            row = cpool.tile([1, CH], F32, tag=f"{name}_row", name=f"{name}_row")
            nc.sync.dma_start(out=row[:], in_=src[:])
            full = cpool.tile([128, CH], F32, tag=f"{name}_full", name=f"{name}_full")
            nc.gpsimd.partition_broadcast(full[:], row[:])
            return full

        gqB = bcast_row("gq", gqd)
        gkB = bcast_row("gk", gkd)
        bqB = bcast_row("bq", bqd)
        bkB = bcast_row("bk", bkd)
        bvB = bcast_row("bv", bvd)

        kwT_sb = []
        for hh in range(HPC):
            t = cpool.tile([128, TW], BF, tag=f"kwT{hh}", name=f"kwT{hh}")
            kwT_sb.append(t)
        cv_sb = [[], []]

        q_sb, k_sb, v_sb = [], [], []
        for j in range(NT):
            q_sb.append(ppool.tile([128, CH], F32, tag=f"q{j}", name=f"q{j}"))
            k_sb.append(ppool.tile([128, CH], F32, tag=f"k{j}", name=f"k{j}"))
            v_sb.append(cpool.tile([128, CH], BF, tag=f"v{j}", name=f"v{j}"))

        rqT_sb = [cpool.tile([128, S], BF, tag=f"rqT{hh}", name=f"rqT{hh}")
                  for hh in range(HPC)]
        oT_sb = [cpool.tile([128, S], BF, tag=f"oT{hh}", name=f"oT{hh}")
                 for hh in range(HPC)]

        GSTART = [0, 5, 10]
        GSIZE = [5, 5, 4]
        ssq, ssk = [], []
        for g in range(3):
            tq = cpool.tile([128, GSIZE[g]], F32, tag=f"ssq{g}", name=f"ssq{g}")
            tk = cpool.tile([128, GSIZE[g]], F32, tag=f"ssk{g}", name=f"ssk{g}")
            nc.gpsimd.memset(tq[:], 0.0)
            nc.gpsimd.memset(tk[:], 0.0)
            ssq.append(tq)
            ssk.append(tk)

        # ---- phase 1: fused QKV projection; ss AllReduce per half ----
        eps_ap = cpool.tile([128, 1], F32, tag="eps_ap")
        nc.gpsimd.memset(eps_ap[:], EPS)
        rs_q, rs_k = [], []

        def emit_group_ar(g):
            n = GSIZE[g]
            nc.sync.dma_start(out=ss_in[g][0], in_=ssq[g][:])
            nc.sync.dma_start(out=ss_in[g][1], in_=ssk[g][:])
            nc.gpsimd.collective_compute(
                "AllReduce", mybir.AluOpType.add, replica_groups=[core_ids],
                ins=[ss_in[g][:]], outs=[ss_out[g][:]])
            ssg = cpool.tile([128, 2 * n], F32, tag=f"ssg{g}", name=f"ssg{g}")
            nc.sync.dma_start(out=ssg[:, 0:n], in_=ss_out[g][0])
            nc.sync.dma_start(out=ssg[:, n:2 * n], in_=ss_out[g][1])
            tmp = wpool.tile([128, 2 * n], F32, tag="rstmp", name=f"rstmp{g}")
            nc.scalar.activation(tmp[:, :2 * n], ssg[:], A.Ln, scale=1.0 / DIM,
                                 bias=eps_ap[:])
            rqk = cpool.tile([128, 2 * n], F32, tag=f"rqk{g}", name=f"rqk{g}")
            nc.scalar.activation(rqk[:, :2 * n], tmp[:, :2 * n], A.Exp, scale=-0.5)
            rs_q.append(rqk[:, 0:n])
            rs_k.append(rqk[:, n:2 * n])

        rq_store = {}
        rope_tr_pool = []

        def rope_dve(j):
            off, sz = S_TILES[j]
            frt = wpool.tile([128, 128], BF, tag="frt", bufs=2, name=f"frt{j}")
            fit = wpool.tile([128, 128], BF, tag="fit", bufs=2, name=f"fit{j}")
            nc.sync.dma_start(out=frt[:sz, :], in_=fr2d[off:off + sz, :])
            nc.sync.dma_start(out=fit[:sz, :], in_=fi2d[off:off + sz, :])
            g = 0 if j < 5 else (1 if j < 10 else 2)
            col = j - GSTART[g]
            for qi, (qk, rscale, gB) in enumerate((
                    (q_sb[j], rs_q[g], gqB), (k_sb[j], rs_k[g], gkB))):
                qn = wpool.tile([128, CH], BF, tag="qn")
                nc.vector.scalar_tensor_tensor(
                    qn[:sz, :], qk[:sz, :], rscale[:sz, col:col + 1], gB[:sz, :],
                    op0=Op.mult, op1=Op.mult)
                q3 = qn[:sz, :].rearrange("p (h c) -> p h c", h=HPC)
                f3r = frt[:sz, :].rearrange("p (h c) -> p h c", h=HPC)
                f3i = fit[:sz, :].rearrange("p (h c) -> p h c", h=HPC)
                qe, qo = q3[:, :, 0:64], q3[:, :, 64:128]
                rq = ppool.tile([128, CH], BF, tag=f"rq{j}_{qi}",
                                name=f"rq{j}_{qi}")
                r3 = rq[:sz, :].rearrange("p (h c) -> p h c", h=HPC)
                t1 = wpool.tile([128, 128], BF, tag="ropet1")
                t2 = wpool.tile([128, 128], BF, tag="ropet2")
                t13 = t1[:sz, :].rearrange("p (h c) -> p h c", h=HPC)
                t23 = t2[:sz, :].rearrange("p (h c) -> p h c", h=HPC)
                nc.vector.tensor_mul(t13, qe, f3r)
                nc.vector.tensor_mul(t23, qo, f3i)
                nc.vector.tensor_sub(r3[:, :, 0:64], t13, t23)
                t3 = wpool.tile([128, 128], BF, tag="ropet1")
                t4 = wpool.tile([128, 128], BF, tag="ropet2")
                t33 = t3[:sz, :].rearrange("p (h c) -> p h c", h=HPC)
                t43 = t4[:sz, :].rearrange("p (h c) -> p h c", h=HPC)
                nc.vector.tensor_mul(t33, qe, f3i)
                nc.vector.tensor_mul(t43, qo, f3r)
                nc.vector.tensor_add(r3[:, :, 64:128], t33, t43)
                rq_store[(j, qi)] = rq

        def rope_tr(j):
            off, sz = S_TILES[j]
            for qi, (dstT, dcol) in enumerate(((rqT_sb, 0), (kwT_sb, S))):
                rq = rq_store[(j, qi)]
                for hh in range(HPC):
                    tp = rope_tr_pool[0].tile([128, 128], BF, tag="tr")
                    nc.tensor.transpose(tp[:, :sz], rq[:sz, D * hh:D * (hh + 1)],
                                        ident[:sz, :sz])
                    nc.vector.tensor_copy(
                        dstT[hh][:, dcol + off:dcol + off + sz], tp[:, :sz])

        with tc.tile_pool(name="pj", bufs=2, space="PSUM") as pj:
            for j, (off, sz) in enumerate(S_TILES):
                if j == 0:
                    xt = xt0
                else:
                    xt = wpool.tile([128, DIM], BF, tag=f"xt{j % 2}", bufs=1,
                                    name=f"xt{j}")
                    nc.sync.dma_start(out=xt[:], in_=xT[j])
                ps = pj.tile([128, 3 * CH], F32, tag="qkv")
                for kk in range(16):
                    nc.tensor.matmul(ps[:sz, 0:512], xt[:, 128 * kk:128 * kk + sz],
                                     wT_sb[kk][:, 0:512],
                                     start=(kk == 0), stop=(kk == 15))
                    nc.tensor.matmul(ps[:sz, 512:768], xt[:, 128 * kk:128 * kk + sz],
                                     wT_sb[kk][:, 512:768],
                                     start=(kk == 0), stop=(kk == 15))
                nc.vector.tensor_add(q_sb[j][:sz, :], ps[:sz, 0:CH], bqB[:sz, :])
                nc.vector.tensor_add(k_sb[j][:sz, :], ps[:sz, CH:2 * CH], bkB[:sz, :])
                nc.vector.tensor_add(v_sb[j][:sz, :], ps[:sz, 2 * CH:3 * CH],
                                     bvB[:sz, :])
                g = 0 if j < 5 else (1 if j < 10 else 2)
                col = j - GSTART[g]
                sq = wpool.tile([128, CH], F32, tag="sqscratch")
                nc.scalar.activation(sq[:sz, :], q_sb[j][:sz, :], A.Square,
                                     accum_out=ssq[g][:sz, col:col + 1])
                sq2 = wpool.tile([128, CH], F32, tag="sqscratch")
                nc.scalar.activation(sq2[:sz, :], k_sb[j][:sz, :], A.Square,
                                     accum_out=ssk[g][:sz, col:col + 1])
                if j == 4:
                    emit_group_ar(0)
                elif j == 9:
                    emit_group_ar(1)
                    for jj in range(0, 5):
                        rope_dve(jj)
            emit_group_ar(2)
            for jj in range(5, 10):
                rope_dve(jj)

        # deferred attention/output constant loads (off the startup critical path)
        for hh in range(HPC):
            nc.sync.dma_start(out=kwT_sb[hh][:, 0:S], in_=ckT[hh])
            big = cpool.tile([128, NT * D], BF, tag=f"cva{hh}", name=f"cva{hh}")
            nc.sync.dma_start(out=big[:], in_=cv[hh])
            cv_sb[hh] = [big[:, j * D:(j + 1) * D] for j in range(NT)]
        woT_sb = []

        def load_woT():
            tpool = tc.alloc_tile_pool(name="tailp", bufs=1)
            for kk in range(16):
                t = tpool.tile([128, DIM], BF, tag=f"woTf{kk}", name=f"woTf{kk}")
                nc.sync.dma_start(out=t[:], in_=woT[128 * kk:128 * (kk + 1), :])
                woT_sb.append(t)
            return tpool

        # ---- phase 2 + 3: transposes interleaved with attention ----
        with tc.tile_pool(name="pat", bufs=2, space="PSUM") as pat:
            rope_tr_pool.append(pat)
            att = {}

            def attn_tiles(hh, jc, tlist):
                jof, jsz = SJ[jc]
                st = att.get((hh, jc))
                if st is None:
                    o_ps = pat.tile([128, 880], F32, tag="o", bufs=1,
                                    name=f"o{hh}_{jc}")
                    den = wpool.tile([128, 880], BF, tag="den", bufs=2,
                                     name=f"den{hh}_{jc}")
                    st = att[(hh, jc)] = (o_ps, den)
                o_ps, den = st
                for ti in tlist:
                    part, j2, toff, tsz = T_TILES[ti]
                    sc = pat.tile([128, 880], F32, tag="sc")
                    nc.tensor.matmul(
                        sc[:tsz, 0:512], kwT_sb[hh][:, toff:toff + tsz],
                        rqT_sb[hh][:, jof:jof + 512], start=True, stop=True)
                    nc.tensor.matmul(
                        sc[:tsz, 512:880], kwT_sb[hh][:, toff:toff + tsz],
                        rqT_sb[hh][:, jof + 512:jof + 880],
                        start=True, stop=True)
                    pT = wpool.tile([128, 880], BF, tag="pT", bufs=4)
                    nc.scalar.activation(pT[:tsz, :], sc[:tsz, :], A.Exp,
                                         scale=SCALE)
                    if ti == 0:
                        nc.vector.tensor_copy(den[:, :], pT[:, :])
                    else:
                        nc.vector.tensor_add(den[:tsz, :], den[:tsz, :],
                                             pT[:tsz, :])
                    vt = (cv_sb[hh][j2][:tsz, :] if part == "c"
                          else v_sb[j2][:tsz, D * hh:D * (hh + 1)])
                    last = ti == len(T_TILES) - 1
                    nc.tensor.matmul(o_ps[:, 0:512], vt, pT[:tsz, 0:512],
                                     start=(ti == 0), stop=last)
                    nc.tensor.matmul(o_ps[:, 512:880], vt, pT[:tsz, 512:880],
                                     start=(ti == 0), stop=last)

            def attn_finish(hh, jc):
                jof, jsz = SJ[jc]
                o_ps, den = att[(hh, jc)]
                denf = wpool.tile([128, 880], F32, tag="denf", bufs=2,
                                  name=f"denf{hh}_{jc}")
                nc.gpsimd.partition_all_reduce(denf[:, :], den[:, :], 128,
                                               bass_isa.ReduceOp.add)
                denr = wpool.tile([128, 880], F32, tag="denr", bufs=2,
                                  name=f"denr{hh}_{jc}")
                nc.vector.reciprocal(denr[:, :jsz], denf[:, :jsz])
                nc.vector.tensor_mul(
                    oT_sb[hh][:, jof:jof + jsz], o_ps[:, :jsz], denr[:, :jsz])

            def emit_a2a(w):
                for hh in range(HPC):
                    nc.sync.dma_start(
                        out=a2a_in[w][:, 128 * hh:128 * (hh + 1), :]
                            .rearrange("d p s -> p d s"),
                        in_=oT_sb[hh][:, 880 * w:880 * (w + 1)]
                            .rearrange("p (d s) -> p d s", s=110))
                nc.gpsimd.collective_compute(
                    "AllToAll", mybir.AluOpType.bypass,
                    replica_groups=[core_ids],
                    ins=[a2a_in[w][:]], outs=[a2a_out[w][:]])

            def wave_y(w):
                otr = tpool.tile([128, 16 * 110], BF, tag=f"otr{w}",
                                 name=f"otr{w}")
                nc.sync.dma_start(
                    out=otr[:].rearrange("p (k s) -> p k s", s=110),
                    in_=a2a_out[w][:].rearrange("d (h p) s -> p (d h) s", p=128))
                yf = wpool.tile([128, DIM], F32, tag="yf", bufs=1, name=f"yf{w}")
                for n in range(4):
                    yp = pat.tile([128, 512], F32, tag="tr", name=f"yp{w}_{n}")
                    for kk in range(16):
                        nc.tensor.matmul(
                            yp[:110, :], otr[:, 110 * kk:110 * (kk + 1)],
                            woT_sb[kk][:, 512 * n:512 * (n + 1)],
                            start=(kk == 0), stop=(kk == 15))
                    nc.scalar.copy(yf[:110, 512 * n:512 * (n + 1)], yp[:110, :])
                nc.sync.dma_start(out=y_out[110 * w:110 * (w + 1), :],
                                  in_=yf[:110, :])

            for j in range(7):
                rope_tr(j)
            attn_tiles(0, 0, range(0, 14))
            for j in range(10, NT):
                rope_dve(j)
            for j in range(7, NT):
                rope_tr(j)
            attn_tiles(0, 0, range(14, 28))
            attn_finish(0, 0)
            ppool.release()
            tpool = load_woT()
            attn_tiles(1, 0, range(28))
            attn_finish(1, 0)
            emit_a2a(0)   # hidden under the second attention half
            attn_tiles(0, 1, range(0, 14))
            wave_y(0)
            attn_tiles(0, 1, range(14, 28))
            attn_finish(0, 1)
            attn_tiles(1, 1, range(28))
            attn_finish(1, 1)
            emit_a2a(1)
            wave_y(1)
        tpool.release()


def _build():
    if "nc" not in _CACHE:
        nc = bacc.Bacc("TRN2", target_bir_lowering=False, debug=False,
                       num_devices=NCORES)
        _emit(nc)
        nc.compile()
        _CACHE["nc"] = nc
    return _CACHE["nc"]


def _make_fcomb(freqs):
    F, H, W = 2, 20, 44
    fr = np.asarray(freqs, np.float32)  # [1024, 64, 2]
    fpart = np.broadcast_to(fr[5:7, None, None, 0:22], (F, H, W, 22, 2))
    hpart = np.broadcast_to(fr[None, 0:H, None, 22:43], (F, H, W, 21, 2))
    wpart = np.broadcast_to(fr[None, None, 0:W, 43:64], (F, H, W, 21, 2))
    return np.concatenate([fpart, hpart, wpart], axis=3).reshape(S, 64, 2)


def kernel(x, wq, bq, wk, bk, wv, bv, wo, bo, gq, gk, freqs, cache_k, cache_v):
    x = np.asarray(x, np.float32)
    wq, wk, wv, wo = (np.asarray(a, np.float32) for a in (wq, wk, wv, wo))
    bq, bk, bv, bo = (np.asarray(a, np.float32) for a in (bq, bk, bv, bo))
    gq, gk = np.asarray(gq, np.float32), np.asarray(gk, np.float32)
    cache_k = np.asarray(cache_k, np.float32)
    cache_v = np.asarray(cache_v, np.float32)

    fcomb = _make_fcomb(freqs)
    fr2 = np.ascontiguousarray(np.tile(fcomb[..., 0], (1, HPC))).astype(BF16)
    fi2 = np.ascontiguousarray(np.tile(fcomb[..., 1], (1, HPC))).astype(BF16)
    # pre-tiled x^T: xT[j, p, kk*128+c] = x[128j+c, 128kk+p]
    xp = np.zeros((NT * 128, DIM), np.float32)
    xp[:S] = x[0]
    xT = np.ascontiguousarray(
        xp.reshape(NT, 128, 16, 128).transpose(0, 3, 2, 1).reshape(NT, 128, DIM)
    ).astype(BF16)

    # de-interleave rope channel pairs within each head: [2c] then [2c+1]
    # (applied consistently to wq/wk rows, their biases/gains, and the
    # transposed k-cache, so attention dot products are unchanged)
    perm = np.concatenate([np.arange(0, D, 2), np.arange(1, D, 2)])
    qk_perm = np.concatenate([h * D + perm for h in range(NH)])
    wqp, wkp = wq[qk_perm], wk[qk_perm]
    bqp, bkp = bq[qk_perm], bk[qk_perm]
    gqp, gkp = gq[qk_perm], gk[qk_perm]
    ck_perm = cache_k[0, WIN0:WIN0 + S][:, :, perm]  # [S, NH, D] channel-permuted

    woT_full = np.ascontiguousarray(wo.T).astype(BF16)  # [DIM, DIM]
    in_maps = []
    for c in range(NCORES):
        hs = slice(CH * c, CH * (c + 1))
        h0 = HPC * c
        wT = np.concatenate([wqp[hs].T, wkp[hs].T, wv[hs].T], axis=1).astype(BF16)
        woTc = woT_full
        ckTc = np.ascontiguousarray(
            ck_perm[:, h0:h0 + HPC, :].transpose(1, 2, 0)
        ).astype(BF16)  # [HPC, D, S]
        # pre-tiled cache-v: cvc[hh, p, j*128+d] = cv_window[128j+p, h, d]
        cw = np.zeros((NT * 128, HPC, D), np.float32)
        cw[:S] = cache_v[0, WIN0:WIN0 + S, h0:h0 + HPC, :]
        cvc = np.ascontiguousarray(
            cw.reshape(NT, 128, HPC, D).transpose(2, 1, 0, 3).reshape(HPC, 128, NT * D)
        ).astype(BF16)
        in_maps.append({
            "xT": xT, "wT": np.ascontiguousarray(wT), "woT": woTc,
            "ckT": ckTc, "cv": cvc, "fr2": fr2, "fi2": fi2,
            "gq": np.ascontiguousarray(gqp[hs])[None, :],
            "gk": np.ascontiguousarray(gkp[hs])[None, :],
            "bq": np.ascontiguousarray(bqp[hs])[None, :],
            "bk": np.ascontiguousarray(bkp[hs])[None, :],
            "bv": np.ascontiguousarray(bv[hs])[None, :],
        })

    nc = _build()
    res = run_bass_kernel_spmd(nc, in_maps, list(range(NCORES)))
    _CACHE["last_result"] = res
    # all-to-all layout: core c returns rows [110c:110c+110] and
    # [880+110c:880+110c+110]
    y = np.empty((S, DIM), np.float32)
    for c in range(NCORES):
        yc = res.results[c]["y"]
        y[110 * c:110 * (c + 1)] = yc[:110]
        y[880 + 110 * c:880 + 110 * (c + 1)] = yc[110:]
    return (y + bo[None, :]).reshape(1, S, DIM).astype(np.float32)
